# revision 41
# baseline (speedup 1.0000x reference)
"""Trainium2 Bass kernel for the Centroid (segment_reduce) problem.

new_centroid = 0.3 * (segment_sum(embed, y) / counts) + 0.7 * centroid
  embed [32768, 1024] f32, y [32768] int64 (0..999), centroid [1000, 1024] f32

Strategy (8 NeuronCores, CLASS-sharded via host-side sort — no collective):
  - host sorts the batch by label; core i gets ALL rows with label in
    [125*i, 125*(i+1)) (125 classes per core). Row counts are ~4096 +- 64,
    padded to a common multiple of 128 (count flag 0 on pad rows) so one
    SPMD program serves all cores. Cores own disjoint classes => zero
    cross-core communication; host unshard is a pure concat.
  - each core's one-hot spans 125 classes -> a single 128-class M-tile:
    fp8 DoubleRow matmuls (pairs of 128-row k-tiles; a trailing odd k-tile
    runs in normal mode) accumulate into ONE PSUM region [128, 1025]
    (cols 0..1023 dims, col 1024 count). Embed is pre-scaled by 0.3 and
    centroid by 0.7 on the host, so finalize is a single fused
    out = psum * (1/count) + cent per 512-col chunk, written as bf16.
  - HW lessons baked in: embed arrives partition-contiguous in 6-7 big
    128-descriptor DMAs alternating the two HWDGE queues (128-desc DMAs
    spread over all 16 SDMA engines; small/odd-count ones don't); the
    centroid rides last on sync; the output leaves in 128-desc bf16
    col-half DMAs on the scalar queue (sync writes drain on few engines),
    each issued right behind its finalize op; ~3.4us of dummy matmuls warm
    the HAM clock gate; ping-pong pair ordering avoids same-weight
    LDWEIGHTS serialization; the count column is accumulated first at the
    end so the reciprocal overlaps the last matmuls.
"""

import numpy as np

import concourse.bacc as bacc
import concourse.mybir as mybir
import concourse.tile as tile
from concourse.bass_utils import run_bass_kernel_spmd

N_CORES = 8
C = 1000  # real classes
CPC = C // N_CORES  # 125 classes owned per core
D = 1024  # embed dim
B = 32768  # total batch
P = 128
W_IN = D + 1  # 1024 dims + count column (col 1024)
W_SB = 1040  # row stride, mult of 16 (DoubleRow step constraint)
FACTOR = 0.3

_F32 = mybir.dt.float32
_BF16 = mybir.dt.bfloat16
_FP8 = mybir.dt.float8e4

_CACHE: dict = {}


def _group_sizes(kp: int) -> list[int]:
    """Pair-tile DMA groups: small first group so the PE starts sooner and
    small last groups so the post-DMA matmul tail is short."""
    if kp <= 4:
        return [1] * kp
    gs = [2]
    rem = kp - 5
    while rem > 0:
        g = min(3, rem)
        gs.append(g)
        rem -= g
    gs += [2, 1]
    return gs


def _build(kt: int):
    """kt = number of 128-row k-tiles per core (pairs run DoubleRow)."""
    kp = kt // 2  # full DoubleRow pairs
    odd = kt % 2  # trailing single k-tile (normal-mode matmul)
    nc = bacc.Bacc(
        "TRN2", target_bir_lowering=False, debug=False, num_devices=N_CORES
    )
    embA = nc.dram_tensor("embA", [P, kt * W_SB], _FP8, kind="ExternalInput").ap()
    lab = nc.dram_tensor("lab", [P, P + kt], _F32, kind="ExternalInput").ap()
    cent = nc.dram_tensor("cent", [P, D], _BF16, kind="ExternalInput").ap()
    # padded to 128 rows: 128-descriptor DMAs spread across all 16 SDMA
    # engines, 125-descriptor ones have been observed on only 5
    out = nc.dram_tensor("out", [P, D], _BF16, kind="ExternalOutput").ap()

    gs = _group_sizes(kp)

    with tile.TileContext(nc) as tc:
        with (
            tc.tile_pool(name="sb", bufs=1) as sb_pool,
            tc.tile_pool(name="psum", bufs=1, space="PSUM") as psum_pool,
        ):
            # PE warm-up: ~3.4us of dummy matmuls so the HAM clock gate
            # (1.2 -> 2.4 GHz) fires as the first real matmuls become ready
            warm = sb_pool.tile([P, 2, 512], _FP8, name="warm", tag="warm")
            nc.vector.memset(warm[:], 0.0)
            ps_w = psum_pool.tile([P, 512], _F32, name="psw", tag="psw")
            for _ in range(8):
                nc.tensor.matmul(
                    ps_w[:],
                    lhsT=warm[:, :, 0:P],
                    rhs=warm[:],
                    start=True,
                    stop=True,
                    perf_mode=mybir.MatmulPerfMode.DoubleRow,
                )

            # tiny labels tensor FIRST on a HWDGE queue so the one-hot build
            # isn't stuck behind the 4.5 MB embed stream
            lab_sb = sb_pool.tile([P, P + kt], _F32, name="lab", tag="lab")
            nc.sync.dma_start(out=lab_sb[:], in_=lab[:])
            iota = lab_sb[:, 0:P]
            y_all = lab_sb[:, P : P + kt]

            # 0.7*centroid (host-prescaled, bf16), needed only at finalize
            cent07 = sb_pool.tile([P, D], _BF16, name="cent", tag="cent")

            # embed groups, sized in k-tiles (pairs; odd tile rides with the
            # last group)
            grp_tiles = []  # (tile, first_ktile, n_ktiles)
            kbase = 0
            for g, sz in enumerate(gs):
                nkt = 2 * sz + (odd if g == len(gs) - 1 else 0)
                t = sb_pool.tile(
                    [P, nkt, W_SB], _FP8, name=f"emb{g}", tag=f"emb{g}"
                )
                dma_eng = nc.scalar if g % 2 == 0 else nc.sync
                off = kbase * W_SB
                dma_eng.dma_start(
                    out=t[:], in_=embA[:, off : off + nkt * W_SB]
                )
                grp_tiles.append((t, kbase, nkt))
                kbase += nkt
            # centroid LAST on the sync queue: needed only at finalize, and
            # keeping it off SWDGE avoids its slow packets stealing SDMA
            # engine time from the embed stream
            nc.sync.dma_start(out=cent07[:], in_=cent[:])

            # per-pair one-hot builds: oh_g[p, k, c] = (y[k*128+p] == c);
            # fine-grained so matmuls pipeline with DMA arrivals
            oh_g = sb_pool.tile([P, kt, P], _FP8, name="ohg", tag="ohg")
            for j in range(kp):
                nc.vector.tensor_tensor(
                    out=oh_g[:, 2 * j : 2 * j + 2, :],
                    in0=iota.unsqueeze(1).broadcast_to([P, 2, P]),
                    in1=y_all[:, 2 * j : 2 * j + 2]
                    .unsqueeze(2)
                    .broadcast_to([P, 2, P]),
                    op=mybir.AluOpType.is_equal,
                )
            if odd:
                nc.vector.tensor_tensor(
                    out=oh_g[:, kt - 1, :],
                    in0=iota,
                    in1=y_all[:, kt - 1 : kt].broadcast_to([P, P]),
                    op=mybir.AluOpType.is_equal,
                )

            ps0 = psum_pool.tile([P, 512], _F32, name="ps0", tag="ps0")
            ps1 = psum_pool.tile([P, 512], _F32, name="ps1", tag="ps1")
            ps2 = psum_pool.tile([P, 1], _F32, name="ps2", tag="ps2")
            chunks = [(ps0, 0, 512), (ps1, 512, 512), (ps2, 1024, 1)]

            def locate(k):
                for t, kb, nk in grp_tiles:
                    if kb <= k < kb + nk:
                        return t, k - kb
                raise AssertionError

            def mm_pair(j, ps, off, n, stop):
                t, l = locate(2 * j)
                nc.tensor.matmul(
                    ps[:],
                    lhsT=oh_g[:, 2 * j : 2 * j + 2, :],
                    rhs=t[:, l : l + 2, off : off + n],
                    start=(j == 0),
                    stop=stop,
                    perf_mode=mybir.MatmulPerfMode.DoubleRow,
                )

            def mm_odd(ps, off, n):
                t, l = locate(kt - 1)
                nc.tensor.matmul(
                    ps[:],
                    lhsT=oh_g[:, kt - 1, :],
                    rhs=t[:, l, off : off + n],
                    start=False,
                    stop=True,
                )

            # ping-pong over pair j and j+1 so consecutive matmuls load
            # DIFFERENT weights (background weight-buffer overlap); same-
            # weight back-to-back LDWEIGHTS serializes on the PE.
            # Final k-tile runs chunk order [count, ps0, ps1] so the
            # reciprocal and first finalize overlap the last matmuls.
            tail_order = [chunks[2], chunks[0], chunks[1]]
            j = 0
            while j < kp:
                last_block = (j + 2 >= kp) and not odd
                if j + 1 < kp:
                    for ps, off, n in (tail_order if last_block else chunks):
                        mm_pair(j, ps, off, n, stop=False)
                        mm_pair(j + 1, ps, off, n, stop=last_block)
                    j += 2
                else:
                    for ps, off, n in (tail_order if last_block else chunks):
                        mm_pair(j, ps, off, n, stop=last_block)
                    j += 1
            if odd:
                for ps, off, n in tail_order:
                    mm_odd(ps, off, n)

            # out = sums * (1/count) + 0.7*centroid   (0.3 folded into embed)
            recip = sb_pool.tile([P, 1], _F32, name="recip", tag="recip")
            nc.vector.reciprocal(recip[:], ps2[:])
            # bf16 output (host upcasts): halves the final HBM write
            out_sb = sb_pool.tile([P, D], _BF16, name="osb", tag="osb")
            for ps, off, n in chunks[:2]:
                nc.vector.scalar_tensor_tensor(
                    out_sb[:, off : off + n],
                    ps[:],
                    recip[:, 0:1],
                    cent07[:, off : off + n],
                    mybir.AluOpType.mult,
                    mybir.AluOpType.add,
                )
            # output DMAs on the SCALAR queue only (sync-queue writes drain
            # on 1-5 SDMA engines; scalar spreads across 16); col-half DMA
            # issued right after its finalize op so it overlaps the second
            # finalize. Rows 125..127 are don't-care padding.
            for _, off, n in chunks[:2]:
                nc.scalar.dma_start(
                    out=out[:, off : off + n], in_=out_sb[:, off : off + n]
                )

    nc.compile()
    return nc


def get_nc(kt: int):
    if kt not in _CACHE:
        _CACHE[kt] = _build(kt)
    return _CACHE[kt]


def prepare(embed: np.ndarray, y: np.ndarray, centroid: np.ndarray):
    """Sort batch by label, shard class-aligned, pad, quantize, lay out
    partition-contiguous. Returns (nc, in_maps)."""
    y = np.asarray(y).astype(np.int64).ravel()
    order = np.argsort(y, kind="stable")
    ys = y[order]
    bounds = np.searchsorted(ys, np.arange(0, C + 1, CPC))
    n_max = int(np.diff(bounds).max())
    kt = max((n_max + P - 1) // P, 2)
    rows = kt * P

    fp8 = mybir.dt.np(_FP8)
    embf = np.asarray(embed, dtype=np.float32) * FACTOR
    cent07 = (np.asarray(centroid, dtype=np.float32) * (1.0 - FACTOR)).astype(
        mybir.dt.np(_BF16)
    )
    one8 = np.float32(1.0).astype(fp8)

    iota_np = np.tile(np.arange(P, dtype=np.float32), (P, 1))
    in_maps = []
    for i in range(N_CORES):
        lo, hi = int(bounds[i]), int(bounds[i + 1])
        n = hi - lo
        idx = order[lo:hi]
        e = np.zeros((rows, W_SB), dtype=fp8)
        e[:n, :D] = embf[idx].astype(fp8)
        e[:n, D] = one8
        # partition-contiguous: embA[p, k*W_SB : (k+1)*W_SB] = row k*128+p
        embA = np.ascontiguousarray(
            e.reshape(kt, P, W_SB).transpose(1, 0, 2).reshape(P, kt * W_SB)
        )
        y_rel = np.zeros(rows, dtype=np.float32)
        y_rel[:n] = (ys[lo:hi] - CPC * i).astype(np.float32)
        lab = np.empty((P, P + kt), dtype=np.float32)
        lab[:, 0:P] = iota_np
        lab[:, P : P + kt] = y_rel.reshape(kt, P).T
        cent_pad = np.zeros((P, D), dtype=mybir.dt.np(_BF16))
        cent_pad[:CPC] = cent07[CPC * i : CPC * (i + 1)]
        in_maps.append({"embA": embA, "lab": lab, "cent": cent_pad})
    return get_nc(kt), in_maps


def assemble(res) -> np.ndarray:
    full = np.concatenate(
        [res.results[i]["out"][:CPC] for i in range(N_CORES)], axis=0
    )
    return np.ascontiguousarray(full).astype(np.float32)


def kernel(embed: np.ndarray, y: np.ndarray, centroid: np.ndarray) -> np.ndarray:
    nc, in_maps = prepare(embed, y, centroid)
    res = run_bass_kernel_spmd(nc, in_maps, core_ids=list(range(N_CORES)))
    return assemble(res)
